# revision 10
# baseline (speedup 1.0000x reference)
"""MultiHeadSelfAttention (qk-LayerNorm variant) on 8 TRN2 NeuronCores.

Problem (B=4, N=2048, C=1024, H=16, D=64, fp32):
    qkv = x @ W_qkv + b_qkv ; q,k,v = split(qkv)
    q = LN(q)*scale ; k = LN(k)          (LN over full C)
    attn = softmax(q_h @ k_h^T) per head ; o = attn @ v_h
    out = concat_heads(o) @ W_proj + b_proj

Sharding: core i handles batch b=i//2 and query-half i%2 (rows 1024*(i%2) ..).
Each core computes K/V for the full sequence of its batch (duplicated across
the half-pair) and attention/proj for its 1024 query rows. No collectives.

Device algorithm (all matmuls fp32r = full-rate fp32 with ~1e-4 relerr):
  P1: QKV^ (x^T staged on host). Q/K produced token-major [t,c] in PSUM,
      LayerNorm applied there (per-partition stats), then PE-transposed to
      [c,t] with gamma/beta (and softmax scale for q) folded into the
      PSUM-evacuation tensor_scalar. K^T and V (augmented with a ones column
      per head for softmax row-sums) staged to DRAM; Q^T stays in SBUF.
  P2: per head-pair flash-style attention: S^T tile = K_h^T.T @ Q_h^T in
      PSUM -> exp on ACT straight into SBUF -> PV accumulation
      (lhsT = [V_h | 1]) giving unnormalized O^T plus row-sums; normalize
      via DVE reciprocal + GpSimd partition_broadcast.
  P3: out = O^T.T @ W_proj + b_proj (bias via K=1 ones-row matmul).
"""
import numpy as np
from contextlib import ExitStack

import concourse.bass as bass
from concourse import bacc
import concourse.tile as tile
import concourse.mybir as mybir
from concourse.masks import make_identity

dt = mybir.dt
AF = mybir.ActivationFunctionType
OP = mybir.AluOpType
ts = bass.ts

B, N, C = 4, 2048, 1024
H, D = 16, 64
NQ = 1024            # query rows per core
SCALE = D ** -0.5
EPS = 1e-6
TT = N // 128        # 16 token tiles (full seq)
TQ = NQ // 128       # 8 token tiles (query half)
CT = C // 128        # 8 channel tiles (= head pairs)
F32R = dt.float32r


def _r(ap):
    return ap.bitcast(F32R)


def build_nc():
    nc = bacc.Bacc()
    xT = nc.dram_tensor("xT", [C, N], dt.float32, kind="ExternalInput")
    xTq = nc.dram_tensor("xTq", [C, NQ], dt.float32, kind="ExternalInput")
    wqkv = nc.dram_tensor("wqkv", [C, 3 * C], dt.float32, kind="ExternalInput")
    wproj = nc.dram_tensor("wproj", [C, C], dt.float32, kind="ExternalInput")
    bqkv = nc.dram_tensor("bqkv", [3 * C], dt.float32, kind="ExternalInput")
    bproj = nc.dram_tensor("bproj", [C], dt.float32, kind="ExternalInput")
    ones128 = nc.dram_tensor("ones128", [128], dt.float32, kind="ExternalInput")
    gq = nc.dram_tensor("gq", [C], dt.float32, kind="ExternalInput")
    bq = nc.dram_tensor("bq", [C], dt.float32, kind="ExternalInput")
    gk = nc.dram_tensor("gk", [C], dt.float32, kind="ExternalInput")
    bk = nc.dram_tensor("bk", [C], dt.float32, kind="ExternalInput")
    out = nc.dram_tensor("out", [NQ, C], dt.float32, kind="ExternalOutput")

    with tile.TileContext(nc) as tc, ExitStack() as top:
        const = top.enter_context(tc.tile_pool(name="const", bufs=1))
        dram = top.enter_context(tc.tile_pool(name="dram", bufs=1, space="DRAM"))
        res = top.enter_context(tc.tile_pool(name="res", bufs=1))

        # ---- constants ----
        ident = const.tile([128, 128], dt.float32)
        make_identity(nc, ident[:])
        ones1 = const.tile([1, 128], F32R)
        nc.sync.dma_start(ones1[:], ones128.rearrange("(o n) -> o n", o=1).bitcast(F32R))
        ones_t = const.tile([128, CT], dt.float32)
        nc.vector.memset(ones_t[:], 1.0)
        eps_t = const.tile([128, 1], dt.float32)
        nc.vector.memset(eps_t[:], EPS)
        gq_t = const.tile([128, CT], dt.float32)
        bq_t = const.tile([128, CT], dt.float32)
        gk_t = const.tile([128, CT], dt.float32)
        bk_t = const.tile([128, CT], dt.float32)
        for t_, d_ in ((gq_t, gq), (bq_t, bq), (gk_t, gk), (bk_t, bk)):
            nc.sync.dma_start(t_[:], d_.rearrange("(ct p) -> p ct", p=128))
        bqkv_t = const.tile([1, 3 * C], F32R)
        nc.sync.dma_start(bqkv_t[:], bqkv.rearrange("(o n) -> o n", o=1).bitcast(F32R))
        bproj_t = const.tile([1, C], F32R)
        nc.sync.dma_start(bproj_t[:], bproj.rearrange("(o n) -> o n", o=1).bitcast(F32R))

        # ---- resident tensors ----
        qnT = res.tile([128, CT, NQ], F32R)    # Q^T, LN'd+scaled [c,t]
        oT = res.tile([128, CT, NQ], F32R)     # O^T unprojected  [c,t]

        # ---- DRAM staging ----
        knT_d = dram.tile([CT, 128, N], dt.float32)          # K^T per pair
        vaug_d = dram.tile([CT, TT, 128, 130], dt.float32)   # [V_h0|1|V_h1|1]

        # ================= P1: QKV + LN + transpose =================
        with ExitStack() as p1:
            wq_p = p1.enter_context(tc.tile_pool(name="wq", bufs=1))
            xt_p = p1.enter_context(tc.tile_pool(name="xt", bufs=2))
            ln_p = p1.enter_context(tc.tile_pool(name="ln", bufs=2))
            st_p = p1.enter_context(tc.tile_pool(name="st", bufs=3))
            ps_p = p1.enter_context(tc.tile_pool(name="ps1", bufs=2, space="PSUM"))
            pst_p = p1.enter_context(tc.tile_pool(name="pst", bufs=2, space="PSUM"))

            def load_w_group(oc_base):
                """[128, CT, C] tile holding W_qkv[:, oc_base:oc_base+C]."""
                w_t = wq_p.tile([128, CT, C], F32R, tag="w_t")
                for kt in range(CT):
                    nc.sync.dma_start(
                        w_t[:, kt, :],
                        wqkv[ts(kt, 128), oc_base:oc_base + C].bitcast(F32R))
                return w_t

            def qkv_psum(ps, x_tile, w_t, oc_base):
                """accumulate x_tile.T @ Wgroup + bias (2 chunks of 512)"""
                for ch in range(2):
                    for kt in range(CT):
                        nc.tensor.matmul(
                            ps[:, ts(ch, 512)],
                            x_tile[:, kt, :],
                            w_t[:, kt, ts(ch, 512)],
                            start=(kt == 0), stop=False,
                            skip_group_check=True)
                    lo = oc_base + ch * 512
                    nc.tensor.matmul(
                        ps[:, ts(ch, 512)], ones1[:],
                        bqkv_t[:, lo:lo + 512],
                        start=False, stop=True, skip_group_check=True)

            def ln_stats(ps_tok):
                """-> (neg_mu, rstd) [128,1] for token-major [128, C] psum."""
                sum_t = ln_p.tile([128, 1], dt.float32, tag="sum")
                nc.vector.tensor_reduce(sum_t[:], ps_tok[:], mybir.AxisListType.X, OP.add)
                neg_mu = ln_p.tile([128, 1], dt.float32, tag="nmu")
                nc.vector.tensor_scalar_mul(neg_mu[:], sum_t[:], -1.0 / C)
                sq = ln_p.tile([128, C], dt.float32, tag="sq")
                ssq = ln_p.tile([128, 1], dt.float32, tag="ssq")
                nc.scalar.activation(sq[:], ps_tok[:], AF.Square, accum_out=ssq[:])
                msq = ln_p.tile([128, 1], dt.float32, tag="msq")
                nc.vector.tensor_tensor(msq[:], neg_mu[:], neg_mu[:], op=OP.mult)
                var = ln_p.tile([128, 1], dt.float32, tag="var")
                nc.vector.tensor_scalar(var[:], ssq[:], 1.0 / C, msq[:],
                                        op0=OP.mult, op1=OP.subtract)
                sv = ln_p.tile([128, 1], dt.float32, tag="sv")
                nc.scalar.activation(sv[:], var[:], AF.Sqrt, bias=eps_t[:])
                rstd = ln_p.tile([128, 1], dt.float32, tag="rstd")
                with nc.allow_low_precision(reason="layernorm rstd"):
                    nc.vector.reciprocal(rstd[:], sv[:])
                return neg_mu, rstd

            def ln_transpose(ps_tok, g_t, b_t, sink):
                """LN-apply token-major psum, PE-transpose 128x128 blocks,
                fold gamma/beta while evacuating; sink(ct, stage_ap)."""
                neg_mu, rstd = ln_stats(ps_tok)
                tok = ln_p.tile([128, C], dt.float32, tag="tok")
                nc.vector.tensor_scalar(tok[:], ps_tok[:], neg_mu[:], rstd[:],
                                        op0=OP.add, op1=OP.mult)
                for ct in range(CT):
                    ps_t = pst_p.tile([128, 128], dt.float32, tag="ps_t")
                    nc.tensor.matmul(ps_t[:], tok[:, ts(ct, 128)], ident[:],
                                     is_transpose=True, start=True, stop=True,
                                     skip_group_check=True)
                    sink(ct, ps_t, g_t[:, ct:ct + 1], b_t[:, ct:ct + 1])

            # ---- K group over full sequence ----
            wk = load_w_group(C)
            for tt in range(TT):
                xt = xt_p.tile([128, CT, 128], F32R, tag="xt")
                for kt in range(CT):
                    nc.sync.dma_start(xt[:, kt, :],
                                      xT[ts(kt, 128), ts(tt, 128)].bitcast(F32R))
                ps_k = ps_p.tile([128, C], dt.float32, tag="ps_k")
                qkv_psum(ps_k, xt, wk, C)

                def k_sink(ct, ps_t, g, b, tt=tt):
                    stg = st_p.tile([128, 128], dt.float32, tag="stg")
                    nc.vector.tensor_scalar(stg[:], ps_t[:], g, b,
                                            op0=OP.mult, op1=OP.add)
                    nc.sync.dma_start(knT_d[ct, :, ts(tt, 128)], stg[:])
                ln_transpose(ps_k, gk_t, bk_t, k_sink)

            # ---- V group over full sequence ----
            wv = load_w_group(2 * C)
            for tt in range(TT):
                xt = xt_p.tile([128, CT, 128], F32R, tag="xt")
                for kt in range(CT):
                    nc.sync.dma_start(xt[:, kt, :],
                                      xT[ts(kt, 128), ts(tt, 128)].bitcast(F32R))
                ps_v = ps_p.tile([128, C], dt.float32, tag="ps_k")
                qkv_psum(ps_v, xt, wv, 2 * C)
                vst = st_p.tile([128, CT, 130], dt.float32, tag="vst")
                for p8 in range(CT):
                    lo = (p8 // 4) * 512 + (p8 % 4) * 128
                    nc.vector.tensor_copy(
                        vst[:, p8, :].rearrange("p (b c) -> p b c", b=2)[:, :, 0:64],
                        ps_v[:, lo:lo + 128].rearrange("p (b c) -> p b c", b=2))
                nc.vector.tensor_copy(vst[:, :, 64], ones_t[:])
                nc.vector.tensor_copy(vst[:, :, 129], ones_t[:])
                nc.sync.dma_start(
                    vaug_d[:, tt, :, :].rearrange("pair p c -> p pair c"), vst[:])

            # ---- Q group over query half ----
            wq = load_w_group(0)
            for tq in range(TQ):
                xt = xt_p.tile([128, CT, 128], F32R, tag="xt")
                for kt in range(CT):
                    nc.sync.dma_start(xt[:, kt, :],
                                      xTq[ts(kt, 128), ts(tq, 128)].bitcast(F32R))
                ps_q = ps_p.tile([128, C], dt.float32, tag="ps_k")
                qkv_psum(ps_q, xt, wq, 0)

                def q_sink(ct, ps_t, g, b, tq=tq):
                    nc.vector.tensor_scalar(qnT[:, ct, ts(tq, 128)], ps_t[:],
                                            g, b, op0=OP.mult, op1=OP.add)
                ln_transpose(ps_q, gq_t, bq_t, q_sink)

        # ================= P2: attention =================
        with ExitStack() as p2:
            kv_p = p2.enter_context(tc.tile_pool(name="kv", bufs=2))
            pt_p = p2.enter_context(tc.tile_pool(name="pt", bufs=4))
            nz_p = p2.enter_context(tc.tile_pool(name="nz", bufs=2))
            ps_s = p2.enter_context(tc.tile_pool(name="ps_s", bufs=4, space="PSUM"))
            ps_o = p2.enter_context(tc.tile_pool(name="ps_o", bufs=2, space="PSUM"))

            for pair in range(CT):
                kp = kv_p.tile([128, N], F32R, tag="kp")
                nc.sync.dma_start(kp[:], knT_d[pair, :, :].bitcast(F32R))
                vp = kv_p.tile([128, TT, 130], F32R, tag="vp")
                nc.sync.dma_start(
                    vp[:],
                    vaug_d[pair, :, :, :].rearrange("kt p c -> p kt c").bitcast(F32R))

                for h2 in range(2):
                    b0 = h2 * 64
                    for qc in range(2):
                        po = ps_o.tile([128, 512], dt.float32, tag="po")
                        for kt in range(TT):
                            pss = ps_s.tile([128, 512], dt.float32, tag="pss")
                            nc.tensor.matmul(
                                pss[:], kp[b0:b0 + 64, ts(kt, 128)],
                                qnT[b0:b0 + 64, pair, ts(qc, 512)],
                                start=True, stop=True, skip_group_check=True)
                            pT = pt_p.tile([128, 512], F32R, tag="pT")
                            nc.scalar.activation(pT[:], pss[:], AF.Exp)
                            nc.tensor.matmul(
                                po[:65, :], vp[:, kt, h2 * 65:h2 * 65 + 65],
                                pT[:],
                                start=(kt == 0), stop=(kt == TT - 1),
                                skip_group_check=True)
                        recip = nz_p.tile([1, 512], dt.float32, tag="recip")
                        with nc.allow_low_precision(reason="softmax denom"):
                            nc.vector.reciprocal(recip[:], po[64:65, :])
                        bc = nz_p.tile([64, 512], dt.float32, tag="bc")
                        nc.gpsimd.partition_broadcast(bc[:], recip[0:1, :])
                        nc.vector.tensor_tensor(
                            oT[b0:b0 + 64, pair, ts(qc, 512)], po[:64, :], bc[:],
                            op=OP.mult)

        # ================= P3: projection =================
        with ExitStack() as p3:
            wp_p = p3.enter_context(tc.tile_pool(name="wp", bufs=1))
            os_p = p3.enter_context(tc.tile_pool(name="os", bufs=3))
            ps_p3 = p3.enter_context(tc.tile_pool(name="ps3", bufs=4, space="PSUM"))

            wp = wp_p.tile([128, CT, C], F32R)
            for kt in range(CT):
                nc.sync.dma_start(wp[:, kt, :],
                                  wproj[ts(kt, 128), :].bitcast(F32R))

            for tq in range(TQ):
                for oc in range(2):
                    ps = ps_p3.tile([128, 512], dt.float32, tag="ps")
                    for ct in range(CT):
                        nc.tensor.matmul(
                            ps[:], oT[:, ct, ts(tq, 128)],
                            wp[:, ct, ts(oc, 512)],
                            start=(ct == 0), stop=False, skip_group_check=True)
                    nc.tensor.matmul(
                        ps[:], ones1[:], bproj_t[:, ts(oc, 512)],
                        start=False, stop=True, skip_group_check=True)
                    ost = os_p.tile([128, 512], dt.float32, tag="ost")
                    nc.vector.tensor_copy(ost[:], ps[:])
                    nc.sync.dma_start(out[ts(tq, 128), ts(oc, 512)], ost[:])

    nc.compile()
    return nc


_NC = None


def _get_nc():
    global _NC
    if _NC is None:
        _NC = build_nc()
    return _NC


def _shard_inputs(inputs):
    x = np.asarray(inputs["x"], dtype=np.float32)
    shared = {
        "wqkv": np.asarray(inputs["W_qkv"], dtype=np.float32),
        "wproj": np.asarray(inputs["W_proj"], dtype=np.float32),
        "bqkv": np.asarray(inputs["b_qkv"], dtype=np.float32),
        "bproj": np.asarray(inputs["b_proj"], dtype=np.float32),
        "ones128": np.ones(128, dtype=np.float32),
        "gq": np.asarray(inputs["q_gamma"], dtype=np.float32) * np.float32(SCALE),
        "bq": np.asarray(inputs["q_beta"], dtype=np.float32) * np.float32(SCALE),
        "gk": np.asarray(inputs["k_gamma"], dtype=np.float32),
        "bk": np.asarray(inputs["k_beta"], dtype=np.float32),
    }
    in_maps = []
    for core in range(8):
        b, half = core // 2, core % 2
        xt = np.ascontiguousarray(x[b].T)
        m = dict(shared)
        m["xT"] = xt
        m["xTq"] = np.ascontiguousarray(xt[:, half * NQ:(half + 1) * NQ])
        in_maps.append(m)
    return in_maps


def kernel(**inputs) -> np.ndarray:
    from concourse.bass_utils import run_bass_kernel_spmd
    nc = _get_nc()
    in_maps = _shard_inputs(inputs)
    res = run_bass_kernel_spmd(nc, in_maps, core_ids=list(range(8)))
    out = np.empty((B, N, C), dtype=np.float32)
    for core in range(8):
        b, half = core // 2, core % 2
        out[b, half * NQ:(half + 1) * NQ, :] = res.results[core]["out"]
    return out


# revision 11
# speedup vs baseline: 461.4555x; 461.4555x over previous
"""MultiHeadSelfAttention (qk-LayerNorm variant) on 8 TRN2 NeuronCores.

Problem (B=4, N=2048, C=1024, H=16, D=64, fp32):
    qkv = x @ W_qkv + b_qkv ; q,k,v = split(qkv)
    q = LN(q)*scale ; k = LN(k)          (LN over full C)
    attn = softmax(q_h @ k_h^T) per head ; o = attn @ v_h
    out = concat_heads(o) @ W_proj + b_proj

Sharding: core i handles batch b=i//2 and query-half i%2 (rows 1024*(i%2) ..).
Each core computes K/V for the full sequence of its batch (duplicated across
the half-pair) and attention/proj for its 1024 query rows. No collectives.

Device algorithm (all matmuls fp32r = full-rate fp32 with ~1e-4 relerr):
  P1: QKV^ (x^T staged on host). Q/K produced token-major [t,c] in PSUM,
      LayerNorm applied there (per-partition stats), then PE-transposed to
      [c,t] with gamma/beta (and softmax scale for q) folded into the
      PSUM-evacuation tensor_scalar. K^T and V (augmented with a ones column
      per head for softmax row-sums) staged to DRAM; Q^T stays in SBUF.
  P2: per head-pair flash-style attention: S^T tile = K_h^T.T @ Q_h^T in
      PSUM -> exp on ACT straight into SBUF -> PV accumulation
      (lhsT = [V_h | 1]) giving unnormalized O^T plus row-sums; normalize
      via DVE reciprocal + GpSimd partition_broadcast.
  P3: out = O^T.T @ W_proj + b_proj (bias via K=1 ones-row matmul).
"""
import numpy as np
from contextlib import ExitStack

import concourse.bass as bass
from concourse import bacc
import concourse.tile as tile
import concourse.mybir as mybir
from concourse.masks import make_identity

dt = mybir.dt
AF = mybir.ActivationFunctionType
OP = mybir.AluOpType
ts = bass.ts

B, N, C = 4, 2048, 1024
H, D = 16, 64
NQ = 1024            # query rows per core
SCALE = D ** -0.5
EPS = 1e-6
TT = N // 128        # 16 token tiles (full seq)
TQ = NQ // 128       # 8 token tiles (query half)
CT = C // 128        # 8 channel tiles (= head pairs)
F32R = dt.float32r


def _r(ap):
    return ap.bitcast(F32R)


def build_nc():
    nc = bacc.Bacc()
    xTt = nc.dram_tensor("xTt", [TT, 128, C], dt.float32, kind="ExternalInput")
    xTqt = nc.dram_tensor("xTqt", [TQ, 128, C], dt.float32, kind="ExternalInput")
    wqkv = nc.dram_tensor("wqkv", [C, 3 * C], dt.float32, kind="ExternalInput")
    wproj = nc.dram_tensor("wproj", [C, C], dt.float32, kind="ExternalInput")
    bqkv = nc.dram_tensor("bqkv", [3 * C], dt.float32, kind="ExternalInput")
    bproj = nc.dram_tensor("bproj", [C], dt.float32, kind="ExternalInput")
    ones128 = nc.dram_tensor("ones128", [128], dt.float32, kind="ExternalInput")
    gq = nc.dram_tensor("gq", [C], dt.float32, kind="ExternalInput")
    bq = nc.dram_tensor("bq", [C], dt.float32, kind="ExternalInput")
    gk = nc.dram_tensor("gk", [C], dt.float32, kind="ExternalInput")
    bk = nc.dram_tensor("bk", [C], dt.float32, kind="ExternalInput")
    out = nc.dram_tensor("out", [NQ, C], dt.float32, kind="ExternalOutput")

    with tile.TileContext(nc) as tc, ExitStack() as top:
        const = top.enter_context(tc.tile_pool(name="const", bufs=1))
        dram = top.enter_context(tc.tile_pool(name="dram", bufs=1, space="DRAM"))
        res = top.enter_context(tc.tile_pool(name="res", bufs=1))

        # ---- constants ----
        ident = const.tile([128, 128], dt.float32)
        make_identity(nc, ident[:])
        ones1 = const.tile([1, 128], F32R)
        nc.sync.dma_start(ones1[:], ones128.rearrange("(o n) -> o n", o=1).bitcast(F32R))
        ones_t = const.tile([128, CT], dt.float32)
        nc.vector.memset(ones_t[:], 1.0)
        eps_t = const.tile([128, 1], dt.float32)
        nc.vector.memset(eps_t[:], EPS)
        gq_t = const.tile([128, CT], dt.float32)
        bq_t = const.tile([128, CT], dt.float32)
        gk_t = const.tile([128, CT], dt.float32)
        bk_t = const.tile([128, CT], dt.float32)
        for t_, d_ in ((gq_t, gq), (bq_t, bq), (gk_t, gk), (bk_t, bk)):
            nc.sync.dma_start(t_[:], d_.rearrange("(ct p) -> p ct", p=128))
        bqkv_t = const.tile([1, 3 * C], F32R)
        nc.sync.dma_start(bqkv_t[:], bqkv.rearrange("(o n) -> o n", o=1).bitcast(F32R))
        bproj_t = const.tile([1, C], F32R)
        nc.sync.dma_start(bproj_t[:], bproj.rearrange("(o n) -> o n", o=1).bitcast(F32R))

        # ---- resident tensors ----
        qnT = res.tile([128, CT, NQ], F32R)    # Q^T, LN'd+scaled [c,t]
        oT = res.tile([128, CT, NQ], F32R)     # O^T unprojected  [c,t]

        # ---- DRAM staging ----
        knT_d = dram.tile([CT, 128, N], dt.float32)          # K^T per pair
        vaug_d = dram.tile([CT, TT, 128, 130], dt.float32)   # [V_h0|1|V_h1|1]

        # ================= P1: QKV + LN + transpose =================
        with ExitStack() as p1:
            wq_p = p1.enter_context(tc.tile_pool(name="wq", bufs=1))
            xt_p = p1.enter_context(tc.tile_pool(name="xt", bufs=2))
            ln_p = p1.enter_context(tc.tile_pool(name="ln", bufs=3))
            st_p = p1.enter_context(tc.tile_pool(name="st", bufs=4))
            ps_p = p1.enter_context(tc.tile_pool(name="ps1", bufs=2, space="PSUM"))
            pst_p = p1.enter_context(tc.tile_pool(name="pst", bufs=2, space="PSUM"))

            def load_w_group(oc_base):
                """[128, CT, C] tile holding W_qkv[:, oc_base:oc_base+C]."""
                w_t = wq_p.tile([128, CT, C], F32R, tag="w_t")
                for kt in range(CT):
                    nc.sync.dma_start(
                        w_t[:, kt, :],
                        wqkv[ts(kt, 128), oc_base:oc_base + C].bitcast(F32R))
                return w_t

            def qkv_psum(ps, x_tile, w_t, oc_base):
                """accumulate x_tile.T @ Wgroup + bias (2 chunks of 512)"""
                for ch in range(2):
                    for kt in range(CT):
                        nc.tensor.matmul(
                            ps[:, ts(ch, 512)],
                            x_tile[:, kt, :],
                            w_t[:, kt, ts(ch, 512)],
                            start=(kt == 0), stop=False,
                            skip_group_check=True)
                    lo = oc_base + ch * 512
                    nc.tensor.matmul(
                        ps[:, ts(ch, 512)], ones1[:],
                        bqkv_t[:, lo:lo + 512],
                        start=False, stop=True, skip_group_check=True)

            def ln_stats(ps_tok):
                """-> (neg_mu, rstd) [128,1] for token-major [128, C] psum."""
                sum_t = ln_p.tile([128, 1], dt.float32, tag="sum")
                nc.vector.tensor_reduce(sum_t[:], ps_tok[:], mybir.AxisListType.X, OP.add)
                neg_mu = ln_p.tile([128, 1], dt.float32, tag="nmu")
                nc.vector.tensor_scalar_mul(neg_mu[:], sum_t[:], -1.0 / C)
                sq = ln_p.tile([128, C], dt.float32, tag="sq")
                ssq = ln_p.tile([128, 1], dt.float32, tag="ssq")
                nc.scalar.activation(sq[:], ps_tok[:], AF.Square, accum_out=ssq[:])
                msq = ln_p.tile([128, 1], dt.float32, tag="msq")
                nc.vector.tensor_tensor(msq[:], neg_mu[:], neg_mu[:], op=OP.mult)
                var = ln_p.tile([128, 1], dt.float32, tag="var")
                nc.vector.tensor_scalar(var[:], ssq[:], 1.0 / C, msq[:],
                                        op0=OP.mult, op1=OP.subtract)
                sv = ln_p.tile([128, 1], dt.float32, tag="sv")
                nc.scalar.activation(sv[:], var[:], AF.Sqrt, bias=eps_t[:])
                rstd = ln_p.tile([128, 1], dt.float32, tag="rstd")
                with nc.allow_low_precision(reason="layernorm rstd"):
                    nc.vector.reciprocal(rstd[:], sv[:])
                return neg_mu, rstd

            def ln_transpose(ps_tok, g_t, b_t, sink):
                """LN-apply token-major psum, PE-transpose 128x128 blocks,
                fold gamma/beta while evacuating; sink(ct, stage_ap)."""
                neg_mu, rstd = ln_stats(ps_tok)
                tok = ln_p.tile([128, C], dt.float32, tag="tok")
                nc.vector.tensor_scalar(tok[:], ps_tok[:], neg_mu[:], rstd[:],
                                        op0=OP.add, op1=OP.mult)
                for ct in range(CT):
                    ps_t = pst_p.tile([128, 128], dt.float32, tag="ps_t")
                    nc.tensor.matmul(ps_t[:], tok[:, ts(ct, 128)], ident[:],
                                     is_transpose=True, start=True, stop=True,
                                     skip_group_check=True)
                    sink(ct, ps_t, g_t[:, ct:ct + 1], b_t[:, ct:ct + 1])

            # ---- K group over full sequence ----
            wk = load_w_group(C)
            for tt in range(TT):
                xt = xt_p.tile([128, CT, 128], F32R, tag="xt")
                nc.sync.dma_start(
                    xt[:].rearrange("p a b -> p (a b)"), xTt[tt].bitcast(F32R))
                ps_k = ps_p.tile([128, C], dt.float32, tag="ps_k")
                qkv_psum(ps_k, xt, wk, C)

                def k_sink(ct, ps_t, g, b, tt=tt):
                    stg = st_p.tile([128, 128], dt.float32, tag="stg")
                    nc.vector.tensor_scalar(stg[:], ps_t[:], g, b,
                                            op0=OP.mult, op1=OP.add)
                    nc.sync.dma_start(knT_d[ct, :, ts(tt, 128)], stg[:])
                ln_transpose(ps_k, gk_t, bk_t, k_sink)

            # ---- V group over full sequence ----
            wv = load_w_group(2 * C)
            for tt in range(TT):
                xt = xt_p.tile([128, CT, 128], F32R, tag="xt")
                nc.sync.dma_start(
                    xt[:].rearrange("p a b -> p (a b)"), xTt[tt].bitcast(F32R))
                ps_v = ps_p.tile([128, C], dt.float32, tag="ps_k")
                qkv_psum(ps_v, xt, wv, 2 * C)
                vst = st_p.tile([128, CT, 130], dt.float32, tag="vst")
                for p8 in range(CT):
                    lo = (p8 // 4) * 512 + (p8 % 4) * 128
                    nc.vector.tensor_copy(
                        vst[:, p8, :].rearrange("p (b c) -> p b c", b=2)[:, :, 0:64],
                        ps_v[:, lo:lo + 128].rearrange("p (b c) -> p b c", b=2))
                nc.vector.tensor_copy(vst[:, :, 64], ones_t[:])
                nc.vector.tensor_copy(vst[:, :, 129], ones_t[:])
                nc.sync.dma_start(
                    vaug_d[:, tt, :, :].rearrange("pair p c -> p pair c"), vst[:])

            # ---- Q group over query half ----
            wq = load_w_group(0)
            for tq in range(TQ):
                xt = xt_p.tile([128, CT, 128], F32R, tag="xt")
                nc.sync.dma_start(
                    xt[:].rearrange("p a b -> p (a b)"), xTqt[tq].bitcast(F32R))
                ps_q = ps_p.tile([128, C], dt.float32, tag="ps_k")
                qkv_psum(ps_q, xt, wq, 0)

                def q_sink(ct, ps_t, g, b, tq=tq):
                    nc.vector.tensor_scalar(qnT[:, ct, ts(tq, 128)], ps_t[:],
                                            g, b, op0=OP.mult, op1=OP.add)
                ln_transpose(ps_q, gq_t, bq_t, q_sink)

        # ================= P2: attention =================
        with ExitStack() as p2:
            kv_p = p2.enter_context(tc.tile_pool(name="kv", bufs=2))
            pt_p = p2.enter_context(tc.tile_pool(name="pt", bufs=4))
            nz_p = p2.enter_context(tc.tile_pool(name="nz", bufs=2))
            ps_s = p2.enter_context(tc.tile_pool(name="ps_s", bufs=2, space="PSUM"))
            ps_o = p2.enter_context(tc.tile_pool(name="ps_o", bufs=2, space="PSUM"))

            for pair in range(CT):
                kp = kv_p.tile([128, N], F32R, tag="kp")
                nc.sync.dma_start(kp[:], knT_d[pair, :, :].bitcast(F32R))
                vp = kv_p.tile([128, TT, 130], F32R, tag="vp")
                nc.sync.dma_start(
                    vp[:],
                    vaug_d[pair, :, :, :].rearrange("kt p c -> p kt c").bitcast(F32R))

                for h2 in range(2):
                    b0 = h2 * 64
                    po = ps_o.tile([128, NQ], dt.float32, tag="po")
                    for kt in range(TT):
                        pss = ps_s.tile([128, NQ], dt.float32, tag="pss")
                        for qc in range(2):
                            nc.tensor.matmul(
                                pss[:, ts(qc, 512)], kp[b0:b0 + 64, ts(kt, 128)],
                                qnT[b0:b0 + 64, pair, ts(qc, 512)],
                                start=True, stop=True, skip_group_check=True)
                        pT = pt_p.tile([128, NQ], F32R, tag="pT")
                        nc.scalar.activation(pT[:], pss[:], AF.Exp)
                        for qc in range(2):
                            nc.tensor.matmul(
                                po[:65, ts(qc, 512)],
                                vp[:, kt, h2 * 65:h2 * 65 + 65],
                                pT[:, ts(qc, 512)],
                                start=(kt == 0), stop=(kt == TT - 1),
                                skip_group_check=True)
                    recip = nz_p.tile([1, NQ], dt.float32, tag="recip")
                    with nc.allow_low_precision(reason="softmax denom"):
                        nc.vector.reciprocal(recip[:], po[64:65, :])
                    bc = nz_p.tile([64, NQ], dt.float32, tag="bc")
                    nc.gpsimd.partition_broadcast(bc[:], recip[0:1, :])
                    nc.vector.tensor_tensor(
                        oT[b0:b0 + 64, pair, :], po[:64, :], bc[:],
                        op=OP.mult)

        # ================= P3: projection =================
        with ExitStack() as p3:
            wp_p = p3.enter_context(tc.tile_pool(name="wp", bufs=1))
            os_p = p3.enter_context(tc.tile_pool(name="os", bufs=3))
            ps_p3 = p3.enter_context(tc.tile_pool(name="ps3", bufs=4, space="PSUM"))

            wp = wp_p.tile([128, CT, C], F32R)
            for kt in range(CT):
                nc.sync.dma_start(wp[:, kt, :],
                                  wproj[ts(kt, 128), :].bitcast(F32R))

            for tq in range(TQ):
                for oc in range(2):
                    ps = ps_p3.tile([128, 512], dt.float32, tag="ps")
                    for ct in range(CT):
                        nc.tensor.matmul(
                            ps[:], oT[:, ct, ts(tq, 128)],
                            wp[:, ct, ts(oc, 512)],
                            start=(ct == 0), stop=False, skip_group_check=True)
                    nc.tensor.matmul(
                        ps[:], ones1[:], bproj_t[:, ts(oc, 512)],
                        start=False, stop=True, skip_group_check=True)
                    ost = os_p.tile([128, 512], dt.float32, tag="ost")
                    nc.vector.tensor_copy(ost[:], ps[:])
                    nc.sync.dma_start(out[ts(tq, 128), ts(oc, 512)], ost[:])

    nc.compile()
    return nc


_NC = None


def _get_nc():
    global _NC
    if _NC is None:
        _NC = build_nc()
    return _NC


def _shard_inputs(inputs):
    x = np.asarray(inputs["x"], dtype=np.float32)
    shared = {
        "wqkv": np.asarray(inputs["W_qkv"], dtype=np.float32),
        "wproj": np.asarray(inputs["W_proj"], dtype=np.float32),
        "bqkv": np.asarray(inputs["b_qkv"], dtype=np.float32),
        "bproj": np.asarray(inputs["b_proj"], dtype=np.float32),
        "ones128": np.ones(128, dtype=np.float32),
        "gq": np.asarray(inputs["q_gamma"], dtype=np.float32) * np.float32(SCALE),
        "bq": np.asarray(inputs["q_beta"], dtype=np.float32) * np.float32(SCALE),
        "gk": np.asarray(inputs["k_gamma"], dtype=np.float32),
        "bk": np.asarray(inputs["k_beta"], dtype=np.float32),
    }
    in_maps = []
    for core in range(8):
        b, half = core // 2, core % 2
        # xTt[tt, p, kt*128+j] = x[b].T[kt*128+p, tt*128+j]
        xt4 = x[b].T.reshape(CT, 128, TT, 128)
        xtt = np.ascontiguousarray(xt4.transpose(2, 1, 0, 3).reshape(TT, 128, C))
        m = dict(shared)
        m["xTt"] = xtt
        m["xTqt"] = np.ascontiguousarray(
            xtt[half * TQ:(half + 1) * TQ])
        in_maps.append(m)
    return in_maps


def kernel(**inputs) -> np.ndarray:
    from concourse.bass_utils import run_bass_kernel_spmd
    nc = _get_nc()
    in_maps = _shard_inputs(inputs)
    res = run_bass_kernel_spmd(nc, in_maps, core_ids=list(range(8)))
    out = np.empty((B, N, C), dtype=np.float32)
    for core in range(8):
        b, half = core // 2, core % 2
        out[b, half * NQ:(half + 1) * NQ, :] = res.results[core]["out"]
    return out
